# revision 75
# baseline (speedup 1.0000x reference)
"""Neural CDE forward pass on 8 Trainium2 NeuronCores (Bass/Tile).

Math (per batch element b):
    z0 = u0 @ Wi + bi                                   [64]
    for t in 0..164:
        h  = relu(z @ W1 + b1)                          [128]
        f  = tanh(h @ W2 + b2)                          [512] -> [64, 8]
        z += einsum('hi,i->h', f, dx_t)                 dx_t = coeffs[t+1]-coeffs[t]
    out_t = z_t @ Wr + br  for every t (166 values)

The scan is chaotic (perturbations amplify ~1.5e4x over 165 steps), so all
matmul paths need ~fp32 accuracy. On the TRN2 PE, fp32 matmuls stream at
4 cyc/col (2 half-speed passes) while fp16 streams at 1 cyc/col. mm2 runs
at fp32-equivalent accuracy in 3 fp16 matmuls via hi/lo splitting:
    h = hh + hl  (hh = fp16(h), hl = fp16(h - hh)), residual ~2^-24 |h|
    h@W2 ~= hh@W2h + hh@W2l + hl@W2h
(numpy-validated: final rel err ~3e-4 vs fp32's 1.6e-4; budget 2e-2).
The einsum-reduce stays fp32: splitting its wide moving operand g costs
more on the vector engines than it saves on the PE (measured), f32r fails
precision (rel err 0.34), and a flipped reduce (g stationary, 66-col S
moving) drowns in per-pass overhead + LDWEIGHTS (all measured on HW).

Measured HW rates that shaped the design (trn2, ramped p-state):
  fp32 matmul 2 passes x ~0.875 ns/col; fp16 1 pass x ~0.42 ns/col;
  LDWEIGHTS ~114 ns per [128,*] stationary (so fp16 matmuls need >= 256
  moving cols to stay exec-bound -> NCHAIN=2, Bc=256, not 4x128);
  Act tanh ~2.1 ns/col + ~175 ns/instr; DVE ~1.4-2 ns/col; GpSimd casts
  ~3-4.7 ns/col (useless for wide splits); GpSimd cannot read PSUM.
The step period (~8.1 us) ~= one chain's serial latency ~= PE work/step:
both are binding, which is why extra hi/lo splits of mm1/reduce (more
hops) and z-in-PSUM accumulation (WAR serialization) all regressed.

Kernel design (per core, batch shard B=512 in NCHAIN=2 chains of Bc=256
on the matmul free dim):
  - State zT [64+2, Bc] fp32 in SBUF per chain; row 64 is constant 1.0
    (turns b1 into a W1 row, so relu needs no bias operand) and row 65
    carries the running readout out_t = z_t @ Wr + br.
  - mm1: h_ps = [W1; b1].T @ zT[0:65]  (fp32, K=65).
  - hh = fp16(relu(h_ps)) on ScalarE; hl = fp16(relu(h_ps) - hh) in one
    fused scalar_tensor_tensor (max 0, subtract) on VectorE.
  - mm2: f_ps[j] = W2h_j.T @ hh + W2l_j.T @ hh + W2h_j.T @ hl, j=0..3
    (12 fp16 matmuls into one PSUM tile).
  - tanh per bank on ScalarE with fused per-partition bias b2_j.
  - einsum: g_j = f_j * dx_rep elementwise fp32 per bank, all on VectorE
    (a GpSimd bank at ~730 ns sat on the serial tail and cost 65 us; DVE
    runs each at ~441 ns), dx_rep[p, b] = dx[b, p % 8] (host
    pre-replicated, DMA streamed);
    e = sum_j S_j'.T @ g_j accumulated over 4 fp32 matmuls in PSUM with
    S_j' [128, 66]: S_j'[p, 16j + p//8] = 1, column 64 = 0 (keeps the
    ones row), column 65 = S_j @ Wr -- the readout accumulates for free.
  - z_new = z_old + e (one VectorE add); row 65 is DMA'd per step.
  - Pipeline: per step, both chains' reduces are emitted first, then both
    mm1s, then both mm2+tanh+g groups; the reduce of (c, t) therefore has
    a full step of slack behind chain c's tanh/g tail.

Baseline (all-fp32, NCHAIN=4) measured 1.56 ms on HW; this version runs
~1.27 ms with rel err 1.9e-3 (budget 2e-2).
"""

import numpy as np

IN_CH = 8
HID = 64
MLP_W = 128
OUT = 1
B_FULL, T = 4096, 166
NSTEP = T - 1
N_CORES = 8
B = B_FULL // N_CORES  # 512
NBANK = 4  # f feature banks of 128
ZR = HID + 2  # z rows: 64 state + ones + out

# tuning knobs
NCHAIN = 2  # independent batch chains interleaved to hide dependency stalls
REPEAT = 1  # run the whole scan REPEAT times (timing amplification only)
SPLIT_MM2 = 1  # fp16 hi/lo split of h/W2 in mm2
SPLIT_MM1 = 0  # fp16 hi/lo split of z/W1 in mm1
RED_F32R = 0  # f32r reduce: FAILS precision (rel err 0.34) and no faster; keep 0
ZPSUM = 0  # z-in-PSUM + accumulate: WAR on state serializes reduce; keep 0
ZSPLIT_ENG = "vector"  # engine for the z hi/lo split ops
FLIP_RED = 0  # reduce with g stationary / S moving (66-col), state batch-major
LAG_TANH = 1  # emit tanh+g of a slot one chain-slot later (decouples Act queue)
G_ON_GPSIMD = 0  # all g-multiplies on DVE (GpSimd op sat on the serial tail)
ASYM = 1  # chain-0 reduce emitted same-step after chain-1 mm2 (NCHAIN=2 only)

_CACHE = {}


def _build_bass():
    from contextlib import ExitStack

    import concourse.tile as tile
    from concourse import bacc, mybir

    f32 = mybir.dt.float32
    f16 = mybir.dt.float16
    AF = mybir.ActivationFunctionType
    ALU = mybir.AluOpType

    nc = bacc.Bacc("TRN2", target_bir_lowering=False, debug=False)

    u0t = nc.dram_tensor("u0t", [IN_CH, B], f32, kind="ExternalInput")
    dxt = nc.dram_tensor("dxt", [NSTEP, 128, B], f32, kind="ExternalInput")
    w1b = nc.dram_tensor("w1b", [HID + 1, MLP_W], f32, kind="ExternalInput")
    w1bh = nc.dram_tensor("w1bh", [HID + 1, MLP_W], f16, kind="ExternalInput")
    w1bl = nc.dram_tensor("w1bl", [HID + 1, MLP_W], f16, kind="ExternalInput")
    w2h = nc.dram_tensor("w2h", [MLP_W, NBANK, 128], f16, kind="ExternalInput")
    w2l = nc.dram_tensor("w2l", [MLP_W, NBANK, 128], f16, kind="ExternalInput")
    w2f = nc.dram_tensor("w2f", [MLP_W, NBANK, 128], f32, kind="ExternalInput")
    b2 = nc.dram_tensor("b2", [128, NBANK], f32, kind="ExternalInput")
    wi = nc.dram_tensor("wi", [IN_CH, ZR], f32, kind="ExternalInput")
    smat_dt = mybir.dt.float32r if RED_F32R else f32
    smat = nc.dram_tensor("smat", [128, NBANK, ZR], smat_dt, kind="ExternalInput")
    ident = nc.dram_tensor("ident", [128, 128], f32, kind="ExternalInput")
    outp = nc.dram_tensor("outp", [T, B], f32, kind="ExternalOutput")

    Bc = B // NCHAIN

    with tile.TileContext(nc) as tc, ExitStack() as ctx:
        const = ctx.enter_context(tc.tile_pool(name="const", bufs=1))
        zpool = ctx.enter_context(tc.tile_pool(name="zpool", bufs=2))
        hpool = ctx.enter_context(tc.tile_pool(name="hpool", bufs=2))
        fpool = ctx.enter_context(tc.tile_pool(name="fpool", bufs=2))
        gpool = ctx.enter_context(tc.tile_pool(name="gpool", bufs=3))
        opool = ctx.enter_context(tc.tile_pool(name="opool", bufs=4))
        dxpool = ctx.enter_context(tc.tile_pool(name="dxpool", bufs=4))
        psum_h = ctx.enter_context(tc.tile_pool(name="psum_h", bufs=2, space="PSUM"))
        psum_f = ctx.enter_context(tc.tile_pool(name="psum_f", bufs=2, space="PSUM"))
        if ZPSUM:
            psum_state = ctx.enter_context(
                tc.tile_pool(name="psum_state", bufs=1, space="PSUM")
            )
        else:
            psum_e = ctx.enter_context(
                tc.tile_pool(name="psum_e", bufs=2, space="PSUM")
            )
        psum_z = psum_h  # z transpose staging shares the h pool (same 1KB size)

        if SPLIT_MM1 or ZPSUM:
            w1bh_sb = const.tile([HID + 1, MLP_W], f16)
            nc.sync.dma_start(w1bh_sb[:], w1bh[:])
            w1bl_sb = const.tile([HID + 1, MLP_W], f16)
            nc.sync.dma_start(w1bl_sb[:], w1bl[:])
        else:
            w1b_sb = const.tile([HID + 1, MLP_W], f32)
            nc.sync.dma_start(w1b_sb[:], w1b[:])
        if SPLIT_MM2:
            w2h_sb = const.tile([MLP_W, NBANK, 128], f16)
            nc.sync.dma_start(w2h_sb[:], w2h[:])
            w2l_sb = const.tile([MLP_W, NBANK, 128], f16)
            nc.sync.dma_start(w2l_sb[:], w2l[:])
        else:
            w2f_sb = const.tile([MLP_W, NBANK, 128], f32)
            nc.sync.dma_start(w2f_sb[:], w2f[:])
        b2_sb = const.tile([128, NBANK], f32)
        nc.sync.dma_start(b2_sb[:], b2[:])
        wi_sb = const.tile([IN_CH, ZR], f32)
        nc.sync.dma_start(wi_sb[:], wi[:])
        s_sb = const.tile([128, NBANK, ZR], smat_dt)
        nc.sync.dma_start(s_sb[:], smat[:])
        u0t_sb = const.tile([IN_CH, B], f32)
        nc.sync.dma_start(u0t_sb[:], u0t[:])
        if FLIP_RED:
            ident_sb = const.tile([128, 128], f32)
            nc.sync.dma_start(ident_sb[:], ident[:])

        z_sb = [None] * NCHAIN
        zh_sb = [None] * NCHAIN
        zl_sb = [None] * NCHAIN
        zT_sb = [None] * NCHAIN  # FLIP_RED state: [128 batch, 2*ZR] per chain
        z_ps = [None] * NCHAIN  # ZPSUM state: persistent [ZR, Bc] PSUM per chain
        dx_tiles = {}
        st = [dict(hh=None, hl=None, h=None, f=None, g=None) for _ in range(NCHAIN)]

        def split_z(c, src_ps, z_c, prev=None):
            """zh = fp16(z[0:65]), zl = fp16(z[0:65] - zh). zh recomputes the
            add with an f16 output so it does not wait on the z32 add."""
            zh = zpool.tile([HID + 1, Bc], f16, tag=f"zh{c}", name=f"zh_sb{c}")
            if prev is None:
                nc.vector.tensor_copy(zh[:], src_ps[0 : HID + 1, :])
            else:
                nc.vector.tensor_add(
                    zh[:], src_ps[0 : HID + 1, :], prev[0 : HID + 1, :]
                )
            zl = zpool.tile([HID + 1, Bc], f16, tag=f"zl{c}", name=f"zl_sb{c}")
            nc.vector.tensor_sub(zl[:], z_c[0 : HID + 1, :], zh[:])
            zh_sb[c], zl_sb[c] = zh, zl

        def out_row(c, t):
            """ZPSUM: readout row PSUM -> SBUF (DMA cannot read PSUM) -> DRAM."""
            cs = slice(c * Bc, (c + 1) * Bc)
            o_sb = opool.tile([2, Bc], f32, name="o_sb")
            nc.scalar.copy(o_sb[:], z_ps[c][HID : HID + 2, :])
            nc.sync.dma_start(outp[t : t + 1, cs], o_sb[1:2, :])

        def split_zp(c):
            """ZPSUM: zh = fp16(z) via Act, zl = fp16(z - zh) via DVE, both
            reading the persistent PSUM state directly."""
            zh = zpool.tile([HID + 1, Bc], f16, tag=f"zh{c}", name=f"zh{c}")
            nc.scalar.copy(zh[:], z_ps[c][0 : HID + 1, :])
            zl = zpool.tile([HID + 1, Bc], f16, tag=f"zl{c}", name=f"zl{c}")
            nc.vector.scalar_tensor_tensor(
                zl[:], z_ps[c][0 : HID + 1, :], 0.0, zh[:], ALU.add, ALU.subtract
            )
            zh_sb[c], zl_sb[c] = zh, zl

        def init_chains():
            if ZPSUM:
                for c in range(NCHAIN):
                    cs = slice(c * Bc, (c + 1) * Bc)
                    zp = psum_state.tile(
                        [ZR, Bc], f32, tag=f"zps{c}", name=f"z_ps{c}"
                    )
                    nc.tensor.matmul(
                        zp[:], wi_sb[:], u0t_sb[:, cs],
                        start=True, stop=False, skip_group_check=True,
                    )
                    z_ps[c] = zp
                    out_row(c, 0)
                    split_zp(c)
                return
            for c in range(NCHAIN):
                cs = slice(c * Bc, (c + 1) * Bc)
                z0_ps = psum_e.tile(
                    [ZR, Bc] if not FLIP_RED else [128, 2 * ZR],
                    f32, tag="e_ps", name=f"z0_ps{c}",
                )
                if FLIP_RED:
                    z0_ps2 = psum_z.tile([128, Bc], f32, tag="h", name="z0_ps2")
                    nc.tensor.matmul(
                        z0_ps2[0:ZR, :], wi_sb[:], u0t_sb[:, cs],
                        start=True, stop=True,
                    )
                    z_c = zpool.tile([ZR, Bc], f32, tag=f"z{c}", name=f"z_sb{c}")
                    nc.vector.tensor_copy(z_c[:], z0_ps2[0:ZR, :])
                    nc.sync.dma_start(outp[0:1, cs], z_c[HID + 1 : HID + 2, :])
                    z_sb[c] = z_c
                    # build the batch-major state zT from z0
                    for half in range(2):
                        nc.tensor.transpose(
                            z0_ps[:, half * ZR : (half + 1) * ZR],
                            z_c[:, half * 128 : (half + 1) * 128],
                            ident_sb[0:ZR, 0:ZR],
                        )
                    zT_c = zpool.tile([128, 2 * ZR], f32, tag=f"zT{c}", name=f"zT{c}")
                    nc.vector.tensor_copy(zT_c[:], z0_ps[:])
                    zT_sb[c] = zT_c
                else:
                    nc.tensor.matmul(
                        z0_ps[:], wi_sb[:], u0t_sb[:, cs], start=True, stop=True
                    )
                    z_c = zpool.tile([ZR, Bc], f32, tag=f"z{c}", name=f"z_sb{c}")
                    nc.vector.tensor_copy(z_c[:], z0_ps[:])
                    nc.sync.dma_start(outp[0:1, cs], z_c[HID + 1 : HID + 2, :])
                    z_sb[c] = z_c
                    if SPLIT_MM1:
                        split_z(c, z0_ps, z_c)

        def frag_trans(c):
            """FLIP_RED: regenerate dim-major z [ZR, Bc] from zT for mm1."""
            z_ps = psum_z.tile([128, Bc], f32, tag="h", name="z_ps")
            for half in range(2):
                nc.tensor.transpose(
                    z_ps[0:ZR, half * 128 : (half + 1) * 128],
                    zT_sb[c][:, half * ZR : (half + 1) * ZR],
                    ident_sb[:],
                )
            z_c = zpool.tile([ZR, Bc], f32, tag=f"z{c}", name=f"z_sb{c}")
            nc.scalar.copy(z_c[:], z_ps[0:ZR, :])
            z_sb[c] = z_c

        def frag_mm1(c, t):
            s = st[c]
            h_ps = psum_h.tile([MLP_W, Bc], f32, tag="h", name="h_ps")
            if SPLIT_MM1 or ZPSUM:
                nc.tensor.matmul(
                    h_ps[:], w1bh_sb[:], zh_sb[c][:], start=True, stop=False
                )
                nc.tensor.matmul(
                    h_ps[:], w1bl_sb[:], zh_sb[c][:], start=False, stop=False
                )
                nc.tensor.matmul(
                    h_ps[:], w1bh_sb[:], zl_sb[c][:], start=False, stop=True
                )
            else:
                nc.tensor.matmul(
                    h_ps[:], w1b_sb[:], z_sb[c][0 : HID + 1, :], start=True, stop=True
                )
            if SPLIT_MM2:
                hh = hpool.tile([MLP_W, Bc], f16, tag="hh", name="hh_sb")
                nc.scalar.activation(hh[:], h_ps[:], AF.Relu)
                hl = hpool.tile([MLP_W, Bc], f16, tag="hl", name="hl_sb")
                nc.vector.scalar_tensor_tensor(
                    hl[:], h_ps[:], 0.0, hh[:], ALU.max, ALU.subtract
                )
                s["hh"], s["hl"] = hh, hl
            else:
                h_sb = hpool.tile([MLP_W, Bc], f32, tag="h32", name="h_sb")
                nc.scalar.activation(h_sb[:], h_ps[:], AF.Relu)
                s["h"] = h_sb

        def frag_mm2(c, t):
            s = st[c]
            f_ps = psum_f.tile([128, NBANK * Bc], f32, name="f_ps")
            s["f_ps"] = f_ps
            if SPLIT_MM2:
                for j in range(NBANK):
                    js = slice(j * Bc, (j + 1) * Bc)
                    nc.tensor.matmul(
                        f_ps[:, js], w2h_sb[:, j, :], s["hh"][:],
                        start=True, stop=False,
                    )
                    nc.tensor.matmul(
                        f_ps[:, js], w2l_sb[:, j, :], s["hh"][:],
                        start=False, stop=False,
                    )
                    nc.tensor.matmul(
                        f_ps[:, js], w2h_sb[:, j, :], s["hl"][:],
                        start=False, stop=True,
                    )
            else:
                for j in range(NBANK):
                    js = slice(j * Bc, (j + 1) * Bc)
                    nc.tensor.matmul(
                        f_ps[:, js], w2f_sb[:, j, :], s["h"][:],
                        start=True, stop=True,
                    )
        def frag_tanh(c, t):
            s = st[c]
            f_ps = s["f_ps"]
            f_sb = fpool.tile([128, NBANK * Bc], f32, name="f_sb")
            for j in range(NBANK):
                js = slice(j * Bc, (j + 1) * Bc)
                nc.scalar.activation(
                    f_sb[:, js], f_ps[:, js], AF.Tanh, bias=b2_sb[:, j : j + 1]
                )
            s["f"] = f_sb

        def frag_g(c, t):
            s = st[c]
            cs = slice(c * Bc, (c + 1) * Bc)
            dx_sb = dx_tiles[t]
            g_dt = mybir.dt.float32r if RED_F32R else f32
            g_sb = gpool.tile([128, NBANK * Bc], g_dt, name="g_sb")
            for j in range(NBANK):
                js = slice(j * Bc, (j + 1) * Bc)
                eng = nc.gpsimd if j < G_ON_GPSIMD else nc.vector
                eng.tensor_mul(g_sb[:, js], s["f"][:, js], dx_sb[:, cs])
            s["g"] = g_sb

        def frag_red(c, t):
            s = st[c]
            cs = slice(c * Bc, (c + 1) * Bc)
            if ZPSUM:
                for j in range(NBANK):
                    js = slice(j * Bc, (j + 1) * Bc)
                    nc.tensor.matmul(
                        z_ps[c][:], s_sb[:, j, :], s["g"][:, js],
                        start=False,
                        stop=(t == NSTEP - 1 and j == NBANK - 1),
                        skip_group_check=True,
                    )
                out_row(c, t + 1)
                if t < NSTEP - 1:
                    split_zp(c)
                return
            if FLIP_RED:
                eT = psum_e.tile([128, 2 * ZR], f32, tag="e_ps", name="eT_ps")
                for half in range(2):
                    hs = slice(half * ZR, (half + 1) * ZR)
                    for j in range(NBANK):
                        gs = slice(
                            j * Bc + half * 128, j * Bc + half * 128 + 128
                        )
                        nc.tensor.matmul(
                            eT[:, hs], s["g"][:, gs], s_sb[:, j, :],
                            start=j == 0, stop=j == NBANK - 1,
                        )
                zT_prev = zT_sb[c]
                zT_sb[c] = zpool.tile(
                    [128, 2 * ZR], f32, tag=f"zT{c}", name=f"zT{c}"
                )
                nc.vector.tensor_add(zT_sb[c][:], eT[:], zT_prev[:])
                for half in range(2):
                    col = half * ZR + HID + 1
                    bs = slice(c * Bc + half * 128, c * Bc + half * 128 + 128)
                    nc.sync.dma_start(
                        outp[t + 1 : t + 2, bs], zT_sb[c][:, col : col + 1]
                    )
                return
            e_ps = psum_e.tile([ZR, Bc], f32, tag="e_ps", name="e_ps")
            f32r = mybir.dt.float32r
            for j in range(NBANK):
                js = slice(j * Bc, (j + 1) * Bc)
                if RED_F32R:
                    nc.tensor.matmul(
                        e_ps[:],
                        s_sb[:, j, :],
                        s["g"][:, js],
                        start=j == 0, stop=j == NBANK - 1,
                    )
                else:
                    nc.tensor.matmul(
                        e_ps[:], s_sb[:, j, :], s["g"][:, js],
                        start=j == 0, stop=j == NBANK - 1,
                    )
            z_prev = z_sb[c]
            z_sb[c] = zpool.tile([ZR, Bc], f32, tag=f"z{c}", name=f"z_sb{c}")
            if SPLIT_MM1:
                # zh first in the DVE queue: mm1's first two matmuls need
                # only zh, so they unblock before the fp32 add even runs
                zh = zpool.tile([HID + 1, Bc], f16, tag=f"zh{c}", name=f"zh{c}")
                nc.vector.tensor_add(
                    zh[:], e_ps[0 : HID + 1, :], z_prev[0 : HID + 1, :]
                )
                nc.vector.tensor_add(z_sb[c][:], e_ps[:], z_prev[:])
                zl = zpool.tile([HID + 1, Bc], f16, tag=f"zl{c}", name=f"zl{c}")
                nc.vector.tensor_sub(zl[:], z_sb[c][0 : HID + 1, :], zh[:])
                zh_sb[c], zl_sb[c] = zh, zl
            else:
                nc.vector.tensor_add(z_sb[c][:], e_ps[:], z_prev[:])
            nc.sync.dma_start(outp[t + 1 : t + 2, cs], z_sb[c][HID + 1 : HID + 2, :])

        def dma_dx(t):
            dx_sb = dxpool.tile([128, B], f32, name="dx_sb")
            nc.sync.dma_start(dx_sb[:], dxt[t])
            dx_tiles[t] = dx_sb
            if t - 4 in dx_tiles:
                del dx_tiles[t - 4]

        for _rep in range(REPEAT):
            init_chains()
            dx_tiles.clear()
            if ASYM:
                # Chain 0's reduce is emitted in the SAME step, after chain
                # 1's mm2: its z-add then sits in the DVE queue ahead of
                # chain 1's g-multiplies, so mm1(0, t+1) unblocks early.
                # Chain 1's reduce stays at the top of the next step.
                for t in range(NSTEP):
                    dma_dx(t)
                    if t > 0:
                        frag_red(1, t - 1)
                    frag_mm1(0, t)
                    frag_mm1(1, t)
                    frag_mm2(0, t)
                    frag_tanh(0, t)
                    frag_g(0, t)
                    frag_mm2(1, t)
                    frag_red(0, t)
                    frag_tanh(1, t)
                    frag_g(1, t)
                frag_red(1, NSTEP - 1)
            else:
                for t in range(NSTEP):
                    dma_dx(t)
                    for c in range(NCHAIN):
                        if t > 0:
                            frag_red(c, t - 1)
                            if FLIP_RED:
                                frag_trans(c)
                    for c in range(NCHAIN):
                        frag_mm1(c, t)
                    for c in range(NCHAIN):
                        frag_mm2(c, t)
                        frag_tanh(c, t)
                        frag_g(c, t)
                for c in range(NCHAIN):
                    frag_red(c, NSTEP - 1)

    nc.compile()
    return nc


def _prep_host(u0, coeffs, W1, b1, W2, b2, Wi, bi, Wr, br):
    f32 = np.float32
    f16 = np.float16

    u0t_full = np.empty((IN_CH, B_FULL), f32)
    u0t_full[: IN_CH - 1] = u0.T
    u0t_full[IN_CH - 1] = 1.0

    dX = (coeffs[:, 1:] - coeffs[:, :-1]).astype(f32)  # [B_FULL, NSTEP, IN_CH]
    dxt_small = np.ascontiguousarray(dX.transpose(1, 2, 0))  # [NSTEP, 8, B_FULL]
    dxt_full = np.tile(dxt_small, (1, 128 // IN_CH, 1))

    # z rows: 0..63 state, 64 ones, 65 readout
    wi_mat = np.zeros((IN_CH, ZR), f32)
    wi_mat[: IN_CH - 1, :HID] = Wi
    wi_mat[IN_CH - 1, :HID] = bi
    wi_mat[IN_CH - 1, HID] = 1.0  # ones row sourced from u0t's 1.0 row
    wi_mat[: IN_CH - 1, HID + 1] = (Wi @ Wr)[:, 0]
    wi_mat[IN_CH - 1, HID + 1] = float(bi @ Wr[:, 0] + br[0])

    # mm1 stationary: [W1; b1] against z rows 0..64 (row 64 is ones)
    w1b_mat = np.empty((HID + 1, MLP_W), f32)
    w1b_mat[:HID] = W1.astype(f32)
    w1b_mat[HID] = b1.astype(f32)
    w1bh = w1b_mat.astype(f16)
    w1bl = (w1b_mat - w1bh.astype(f32)).astype(f16)

    w2_banks = np.ascontiguousarray(W2.astype(f32).reshape(MLP_W, NBANK, 128))
    w2h = w2_banks.astype(f16)
    w2l = (w2_banks - w2h.astype(f32)).astype(f16)
    b2_banks = np.ascontiguousarray(b2.astype(f32).reshape(NBANK, 128).T)

    p = np.arange(128)
    s_full = np.zeros((128, NBANK, ZR), f32)
    for j in range(NBANK):
        s_full[p, j, 16 * j + p // IN_CH] = 1.0
        s_full[p, j, HID + 1] = Wr[16 * j + p // IN_CH, 0]

    return {
        "u0t": u0t_full,
        "dxt": dxt_full,
        "w1b": w1b_mat,
        "w1bh": w1bh,
        "w1bl": w1bl,
        "w2h": w2h,
        "w2l": w2l,
        "w2f": w2_banks,
        "b2": b2_banks.astype(f32),
        "wi": wi_mat,
        "smat": s_full,
        "ident": np.eye(128, dtype=f32),
    }


def _make_in_maps(full):
    in_maps = []
    for c in range(N_CORES):
        sl = slice(c * B, (c + 1) * B)
        in_maps.append(
            {
                "u0t": np.ascontiguousarray(full["u0t"][:, sl]),
                "dxt": np.ascontiguousarray(full["dxt"][:, :, sl]),
                "w1b": full["w1b"],
                "w1bh": full["w1bh"],
                "w1bl": full["w1bl"],
                "w2h": full["w2h"],
                "w2l": full["w2l"],
                "w2f": full["w2f"],
                "b2": full["b2"],
                "wi": full["wi"],
                "smat": full["smat"],
                "ident": full["ident"],
            }
        )
    return in_maps


def kernel(u0, coeffs, W1, b1, W2, b2, Wi, bi, Wr, br):
    from concourse.bass_utils import run_bass_kernel_spmd

    full = _prep_host(
        np.asarray(u0, np.float32), np.asarray(coeffs, np.float32),
        np.asarray(W1, np.float32), np.asarray(b1, np.float32),
        np.asarray(W2, np.float32), np.asarray(b2, np.float32),
        np.asarray(Wi, np.float32), np.asarray(bi, np.float32),
        np.asarray(Wr, np.float32).reshape(HID, OUT),
        np.asarray(br, np.float32).reshape(OUT),
    )
    in_maps = _make_in_maps(full)

    if "nc" not in _CACHE:
        _CACHE["nc"] = _build_bass()
    nc = _CACHE["nc"]

    res = run_bass_kernel_spmd(nc, in_maps, core_ids=list(range(N_CORES)))
    _CACHE["last_res"] = res
    outs = res.results

    out_full = np.empty((B_FULL, T, OUT), np.float32)
    for c in range(N_CORES):
        out_full[c * B : (c + 1) * B, :, 0] = outs[c]["outp"].T
    return out_full
